# revision 22
# baseline (speedup 1.0000x reference)
"""Trainium2 Bass kernel for nn_Attention_10015863734775.

Multi-head causal attention (16 heads, d_model 2048, d_head 128, seq 2048,
batch 1) with llama-style interleaved RoPE and a signed-softmax:
    attn_w = sign(s) * exp(|s| - max|s|);  attn = attn_w / (sum|attn_w| + 1e-6)
The max-subtraction cancels in the normalization, so the device computes
attn = sign(s)exp(|s|) / sum exp(|s|).

Sharding: 2 heads per NeuronCore (8 cores). Each core receives the full
transposed residual X^T plus its head slices of W_Q/K/V/W_O and computes a
partial output projection outT_c[m, s] in bf16; the host sums the 8 partials
in f32, transposes, and adds b_O.

Precision split (empirically validated): q/k-path rounding amplifies ~15x
into the output (correlated per-row errors accumulate over the d_head
contraction), so X, W_Q/K/V, RoPE tables and qrot/krot stay f32/f32r.
Value-path tensors (e2, w, v, znt, W_O, output staging) are bf16 — each
contributes only ~0.2% rel-l2 — enabling bf16 z/denominator/output matmuls
(same 1 cycle/row as f32r) and moving the w=sign*e2 multiply to the
(otherwise idle, PSUM-incapable) GPSIMD engine.

Engine budget per score chunk: abs+sign (PSUM-reading: DVE/ACT), exp (ACT,
bf16 out), mult (GPSIMD/DVE-2x), causal mask adds (GPSIMD on SBUF |s|).
Denominator: ones-bf16 matmul accumulated per row; reciprocal_approx_fast
(~5x faster than reciprocal) + gpsimd partition_broadcast for 1/d.
"""

import math

import numpy as np

S = 2048          # sequence length
D = 2048          # d_model
DH = 128          # d_head
NH = 16           # total heads
NC = 8            # neuron cores
HPC = NH // NC    # heads per core (2)
ST = 512          # seq tile (matmul free dim / one PSUM bank)
NST = S // ST     # 4 seq tiles
NDC = D // 128    # 16 contraction chunks
NKC = S // 128    # 16 k chunks
C_SCALE = 1.0 / math.sqrt(float(DH))
LN2 = math.log(2.0)
MASK_NEG = -1.0e5

_CACHE = {}


def _build_program():
    import concourse.tile as tile
    from concourse import bacc, mybir

    F32 = mybir.dt.float32
    F32R = mybir.dt.float32r
    BF16 = mybir.dt.bfloat16
    U32 = mybir.dt.uint32
    AF = mybir.ActivationFunctionType
    ALU = mybir.AluOpType

    nc = bacc.Bacc("TRN2", target_bir_lowering=False, debug=False, num_devices=NC)

    xt_d = nc.dram_tensor("xt", [D, S], F32, kind="ExternalInput").ap()
    wall_d = nc.dram_tensor("wall", [NDC, 6, 128, DH], F32, kind="ExternalInput").ap()
    wo_d = nc.dram_tensor("wo", [HPC, DH, D], BF16, kind="ExternalInput").ap()
    bq_d = nc.dram_tensor("bq", [HPC, DH, 1], F32, kind="ExternalInput").ap()
    bk_d = nc.dram_tensor("bk", [HPC, DH, 1], F32, kind="ExternalInput").ap()
    bv_d = nc.dram_tensor("bv", [HPC, DH, 1], F32, kind="ExternalInput").ap()
    cos_d = nc.dram_tensor("cost", [DH, S], F32, kind="ExternalInput").ap()
    sin_d = nc.dram_tensor("sint", [DH, S], F32, kind="ExternalInput").ap()
    msk_d = nc.dram_tensor("maskneg", [128, 896], F32, kind="ExternalInput").ap()
    pt_d = nc.dram_tensor("pt", [128, 128], F32, kind="ExternalInput").ap()
    idb_d = nc.dram_tensor("identb", [128, 128], BF16, kind="ExternalInput").ap()
    ocb_d = nc.dram_tensor("onescolb", [128, 1], BF16, kind="ExternalInput").ap()
    out_d = nc.dram_tensor("outt", [D, S], BF16, kind="ExternalOutput").ap()

    with tile.TileContext(nc) as tc:
        with tc.tile_pool(name="persist", bufs=1) as pp:
            # ---------------- persistent SBUF ------------------------------
            bqs, bks, bvs = [], [], []
            qrot, krot, vb_sb, zntb = [], [], [], []
            for h in range(HPC):
                for lst, dd, nm in ((bqs, bq_d, "bq"), (bks, bk_d, "bk"), (bvs, bv_d, "bv")):
                    bt = pp.tile([DH, 1], F32, tag=f"{nm}{h}")
                    nc.scalar.dma_start(bt[:], dd[h])
                    lst.append(bt)
                qrot.append(pp.tile([DH, S], F32R, tag=f"qrot{h}", name=f"qrot{h}"))
                krot.append(pp.tile([DH, S], F32R, tag=f"krot{h}", name=f"krot{h}"))
                vb_sb.append(pp.tile([128, NKC, DH], BF16, tag=f"vb{h}", name=f"vb{h}"))
                zntb.append(pp.tile([DH, S], BF16, tag=f"zntb{h}", name=f"zntb{h}"))
            wo_sb = [pp.tile([DH, D], BF16, tag=f"wo{h}", name=f"wo{h}")
                     for h in range(HPC)]
            msk_sb = pp.tile([128, 896], F32, tag="msk")
            pt_sb = pp.tile([128, 128], F32R, tag="pt")
            nc.scalar.dma_start(pt_sb[:], pt_d[:].bitcast(F32R))
            idb_sb = pp.tile([128, 128], BF16, tag="identb")
            nc.scalar.dma_start(idb_sb[:], idb_d[:])
            ocb_sb = pp.tile([128, 1], BF16, tag="onescolb")
            nc.scalar.dma_start(ocb_sb[:], ocb_d[:])
            cabs_sb = pp.tile([128, 1], U32, tag="cabs")
            nc.vector.memset(cabs_sb[:], 0x7FFFFFFF)
            csgn_sb = pp.tile([128, 1], U32, tag="csgn")
            nc.vector.memset(csgn_sb[:], 0x80000000)
            cone_sb = pp.tile([128, 1], U32, tag="cone")
            nc.vector.memset(cone_sb[:], 0x3F800000)
            ln2_sb = pp.tile([128, 1], F32, tag="ln2")
            nc.vector.memset(ln2_sb[:], LN2)
            zero_sb = pp.tile([128, 1], F32, tag="zero")
            nc.vector.memset(zero_sb[:], 0.0)
            import os as _os
            _nonce = float(int(_os.environ.get("KBUILD_NONCE", "0")))
            nonce_sb = pp.tile([128, 1], F32, tag="nonce")
            nc.vector.memset(nonce_sb[:], _nonce)

            # greedy engine balancing for elementwise passes
            load = {"dve": 0.0, "act": 0.0, "gp": 0.0}

            def pick(*opts):
                e, c = min(opts, key=lambda ec: load[ec[0]] + ec[1])
                load[e] += c
                return e

            # ---------------- Phase A: projections + RoPE + V transpose ----
            with tc.tile_pool(name="aphase", bufs=1) as ap_, \
                 tc.tile_pool(name="axt", bufs=4) as axt, \
                 tc.tile_pool(name="aev", bufs=6) as aev, \
                 tc.tile_pool(name="arope", bufs=6) as arp, \
                 tc.tile_pool(name="psA", bufs=1, space="PSUM") as psA, \
                 tc.tile_pool(name="psShuf", bufs=1, space="PSUM") as psSh, \
                 tc.tile_pool(name="psVtr", bufs=1, space="PSUM") as psVt:
                wall_sb = ap_.tile([128, NDC, 6, DH], F32R, tag="wall", name="wall_sb")
                widx = {("q", 0): 0, ("q", 1): 1, ("k", 0): 2, ("k", 1): 3,
                        ("v", 0): 4, ("v", 1): 5}
                cos_sb = ap_.tile([DH, S], F32, tag="cos")
                sin_sb = ap_.tile([DH, S], F32, tag="sin")

                a_deferred = []
                for st in range(NST):
                    ssl = slice(st * ST, (st + 1) * ST)
                    acc = {}
                    for key in ("q", "k", "v"):
                        for h in range(HPC):
                            acc[(key, h)] = psA.tile([128, ST], F32, tag=f"acc{key}{h}",
                                                     name=f"acc{key}{h}_{st}")
                    for dc in range(NDC):
                        if st == 0:
                            nc.gpsimd.dma_start(
                                wall_sb[:, dc, :, :],
                                wall_d[dc].rearrange("i p e -> p i e").bitcast(F32R))
                        xt_t = axt.tile([128, ST], F32R, tag="xt")
                        nc.sync.dma_start(
                            xt_t[:],
                            xt_d[dc * 128:(dc + 1) * 128, ssl].bitcast(F32R))
                        if st == 0 and dc == 4:
                            nc.scalar.dma_start(cos_sb[:], cos_d[:])
                            nc.scalar.dma_start(sin_sb[:], sin_d[:])
                        for key in ("q", "k", "v"):
                            for h in range(HPC):
                                nc.tensor.matmul(
                                    acc[(key, h)][:], wall_sb[:, dc, widx[(key, h)], :],
                                    xt_t[:], start=(dc == 0), stop=(dc == NDC - 1))
                        if a_deferred and dc % 2 == 1:
                            a_deferred.pop(0)()

                    def make_rope(key, h, st, ssl, x_sb):
                        def run():
                            dst = (qrot if key == "q" else krot)[h]
                            shuf = psSh.tile([128, ST], F32, tag="shuf",
                                             name=f"sh{key}{h}_{st}")
                            nc.tensor.matmul(shuf[:], pt_sb[:], x_sb[:],
                                             start=True, stop=True)
                            t1 = arp.tile([128, ST], F32, tag="t1", name=f"t1{key}{h}_{st}")
                            nc.gpsimd.tensor_tensor(t1[:], x_sb[:].bitcast(F32),
                                                    cos_sb[:, ssl], ALU.mult)
                            t2 = arp.tile([128, ST], F32, tag="t2", name=f"t2{key}{h}_{st}")
                            nc.vector.tensor_tensor(t2[:], shuf[:], sin_sb[:, ssl], ALU.mult)
                            nc.vector.tensor_tensor(dst[:, ssl], t1[:], t2[:], ALU.add)
                        return run

                    def make_vtr(h, st, vt_sb):
                        def run():
                            for sc in range(ST // 128):
                                vtr = psVt.tile([128, 128], BF16, tag="vtr",
                                                name=f"vtr{h}_{st}_{sc}")
                                nc.tensor.transpose(vtr[:],
                                                    vt_sb[:, sc * 128:(sc + 1) * 128],
                                                    idb_sb[:])
                                if pick(("dve", 0.17), ("act", 0.16)) == "dve":
                                    nc.vector.tensor_copy(vb_sb[h][:, st * 4 + sc, :],
                                                          vtr[:])
                                else:
                                    nc.scalar.activation(vb_sb[h][:, st * 4 + sc, :],
                                                         vtr[:], AF.Identity,
                                                         bias=zero_sb[:])
                        return run

                    for key in ("q", "k"):
                        for h in range(HPC):
                            bias = (bqs if key == "q" else bks)[h]
                            x_sb = aev.tile([128, ST], F32R, tag="ev", bufs=8,
                                            name=f"ev{key}{h}_{st}")
                            nc.scalar.activation(x_sb[:], acc[(key, h)][:], AF.Identity,
                                                 bias=bias[:])
                            a_deferred.append(make_rope(key, h, st, ssl, x_sb))
                    for h in range(HPC):
                        vt_sb = aev.tile([128, ST], BF16, tag="evv", bufs=4,
                                         name=f"evv{h}_{st}")
                        nc.scalar.activation(vt_sb[:], acc[("v", h)][:], AF.Identity,
                                             bias=bvs[h][:])
                        a_deferred.append(make_vtr(h, st, vt_sb))
                while a_deferred:
                    a_deferred.pop(0)()

            # deferred non-critical DMAs (needed in B/C only)
            nc.scalar.dma_start(msk_sb[:], msk_d[:])
            for h in range(HPC):
                nc.scalar.dma_start(wo_sb[h][:], wo_d[h])

            # ---------------- Phases B + C --------------------------------
            with tc.tile_pool(name="bwork", bufs=6) as bw, \
                 tc.tile_pool(name="bbf", bufs=4) as bb, \
                 tc.tile_pool(name="bsmall", bufs=3) as bsm, \
                 tc.tile_pool(name="psS", bufs=2, space="PSUM") as psS, \
                 tc.tile_pool(name="psB", bufs=1, space="PSUM") as psB, \
                 tc.tile_pool(name="psD", bufs=1, space="PSUM") as psD:
                # PSUM banks: psS 2 ("s") + psB 4 ("z0","z1", "co"x2) + psD 2 ("d0","d1") = 8

                dma_engines = [nc.sync, nc.scalar, nc.gpsimd]

                def emit_scores(j, kc, h):
                    jj = kc - 4 * j
                    lo = jj * 128 if 0 <= jj < 4 else 0
                    pss = psS.tile([128, ST], F32, tag="s", name=f"s{h}_{j}_{kc}")
                    nc.tensor.matmul(pss[:, lo:], krot[h][:, kc * 128:(kc + 1) * 128],
                                     qrot[h][:, j * ST + lo:(j + 1) * ST],
                                     start=True, stop=True)
                    return {"j": j, "kc": kc, "h": h, "lo": lo, "pss": pss}

                def emit_elem(stt):
                    j, kc, h, lo, pss = stt["j"], stt["kc"], stt["h"], stt["lo"], stt["pss"]
                    jj = kc - 4 * j
                    lsl = slice(lo, ST)
                    f = (ST - lo) / ST
                    a = bw.tile([128, ST], F32, tag="a", name=f"a{h}_{j}_{kc}")
                    if pick(("dve", 0.60 * f), ("act", 0.52 * f)) == "dve":
                        nc.vector.tensor_scalar(a[:, lsl].bitcast(U32),
                                                pss[:, lsl].bitcast(U32),
                                                cabs_sb[:], None, ALU.bitwise_and)
                        exp_scale = C_SCALE
                    else:
                        nc.scalar.activation(a[:, lsl], pss[:, lsl], AF.Abs,
                                             bias=zero_sb[:], scale=C_SCALE)
                        exp_scale = 1.0
                    if 0 <= jj < 4:
                        # causal mask on |s|: -1e5 -> exp == 0 (gpsimd, SBUF)
                        load["gp"] += 0.13
                        nc.gpsimd.tensor_tensor(a[:, lo:lo + 128], a[:, lo:lo + 128],
                                                msk_sb[:, 384:512], ALU.add)
                    sg_bf = None
                    sg_f = None
                    if pick(("dve", 0.60 * f), ("act", 0.52 * f)) == "dve":
                        sg_f = bw.tile([128, ST], F32, tag="sg", name=f"sg{h}_{j}_{kc}")
                        nc.vector.tensor_scalar(sg_f[:, lsl].bitcast(U32),
                                                pss[:, lsl].bitcast(U32),
                                                csgn_sb[:], cone_sb[:],
                                                ALU.bitwise_and, ALU.bitwise_or)
                    else:
                        sg_bf = bb.tile([128, ST], BF16, tag="sgb", name=f"sgb{h}_{j}_{kc}")
                        nc.scalar.activation(sg_bf[:, lsl], pss[:, lsl], AF.Sign,
                                             bias=zero_sb[:])
                    e2 = bb.tile([128, ST], BF16, tag="e2", name=f"e2{h}_{j}_{kc}")
                    load["act"] += 0.52 * f
                    nc.scalar.activation(e2[:, lsl], a[:, lsl], AF.Exp,
                                         bias=ln2_sb[:], scale=exp_scale)
                    w = bb.tile([128, ST], BF16, tag="w", name=f"w{h}_{j}_{kc}")
                    if sg_bf is not None:
                        # all-bf16 SBUF multiply: DVE 2x-eligible
                        if pick(("dve", 0.32 * f), ("gp", 0.46 * f)) == "dve":
                            nc.vector.tensor_tensor(w[:, lsl], sg_bf[:, lsl],
                                                    e2[:, lsl], ALU.mult)
                        else:
                            nc.gpsimd.tensor_tensor(w[:, lsl], sg_bf[:, lsl],
                                                    e2[:, lsl], ALU.mult)
                    else:
                        if pick(("dve", 0.60 * f), ("gp", 0.46 * f)) == "dve":
                            nc.vector.tensor_tensor(w[:, lsl], sg_f[:, lsl],
                                                    e2[:, lsl], ALU.mult)
                        else:
                            nc.gpsimd.tensor_tensor(w[:, lsl], sg_f[:, lsl],
                                                    e2[:, lsl], ALU.mult)
                    stt["e2"], stt["w"] = e2, w

                def emit_zd(stt, psz, psd_t, nkc_):
                    j, kc, h, lo = stt["j"], stt["kc"], stt["h"], stt["lo"]
                    lsl = slice(lo, ST)
                    nc.tensor.matmul(psd_t[:, lsl], ocb_sb[:], stt["e2"][:, lsl],
                                     start=(kc == 0), stop=(kc == nkc_ - 1))
                    nc.tensor.matmul(psz[:, lsl], vb_sb[h][:, kc, :], stt["w"][:, lsl],
                                     start=(kc == 0), stop=(kc == nkc_ - 1))

                def finalize(j, h, psz, psd_t):
                    jsl = slice(j * ST, (j + 1) * ST)
                    r_sb = bsm.tile([1, ST], F32, tag="rsb", name=f"r{h}_{j}")
                    load["dve"] += 0.75
                    nc.vector.reciprocal_approx_fast(r_sb[:], psd_t[:])
                    rb_sb = bw.tile([128, ST], F32, tag="a", name=f"rbs{h}_{j}")
                    load["gp"] += 0.55
                    nc.gpsimd.partition_broadcast(rb_sb[:], r_sb[:])
                    load["dve"] += 0.60
                    nc.vector.tensor_tensor(zntb[h][:, jsl], psz[:], rb_sb[:], ALU.mult)

                def make_C(j):
                    jsl = slice(j * ST, (j + 1) * ST)
                    acts = []
                    for mc in range(16):
                        def run(mc=mc):
                            co = psB.tile([128, ST], F32, tag="co", bufs=2,
                                          name=f"o{j}_{mc}")
                            for h in range(HPC):
                                nc.tensor.matmul(co[:],
                                                 wo_sb[h][:, mc * 128:(mc + 1) * 128],
                                                 zntb[h][:, jsl],
                                                 start=(h == 0), stop=(h == HPC - 1))
                            o_sb = bb.tile([128, ST], BF16, tag="osb", bufs=3,
                                           name=f"osb{j}_{mc}")
                            if pick(("dve", 0.60), ("act", 0.52)) == "act":
                                nc.scalar.activation(o_sb[:], co[:], AF.Identity,
                                                     bias=zero_sb[:])
                            else:
                                nc.vector.tensor_copy(o_sb[:], co[:])
                            dma_engines[mc % 3].dma_start(
                                out_d[mc * 128:(mc + 1) * 128, jsl], o_sb[:])
                        acts.append(run)
                    return acts

                c_deferred = []
                units = [(j, kc, h) for j in range(NST)
                         for kc in range(4 * (j + 1)) for h in range(HPC)]
                LOOK = 2
                ZLAG = 2
                inflight = {}
                psz = {}
                psd_t = {}

                def stage_zd(u):
                    stt = inflight.pop(u)
                    j, h = stt["j"], stt["h"]
                    nkc_ = 4 * (j + 1)
                    if stt["kc"] == 0:
                        psd_t[(j, h)] = psD.tile([1, ST], F32, tag=f"d{h}", name=f"d{h}_{j}")
                        psz[(j, h)] = psB.tile([128, ST], F32, tag=f"z{h}",
                                               name=f"z{h}_{j}")
                    emit_zd(stt, psz[(j, h)], psd_t[(j, h)], nkc_)
                    if stt["kc"] == nkc_ - 1:
                        finalize(j, h, psz.pop((j, h)), psd_t.pop((j, h)))
                        if h == HPC - 1:
                            c_deferred.extend(make_C(j))

                for u, (j, kc, h) in enumerate(units):
                    inflight[u] = emit_scores(j, kc, h)
                    if u >= LOOK:
                        emit_elem(inflight[u - LOOK])
                    if u >= LOOK + ZLAG:
                        stage_zd(u - LOOK - ZLAG)
                    if c_deferred:
                        c_deferred.pop(0)()
                n = len(units)
                for u in range(n - LOOK, n):
                    emit_elem(inflight[u])
                for u in range(n - LOOK - ZLAG, n):
                    stage_zd(u)
                    if c_deferred:
                        c_deferred.pop(0)()
                while c_deferred:
                    c_deferred.pop(0)()
    nc.compile()
    return nc


def _host_constants():
    import ml_dtypes
    BFNP = ml_dtypes.bfloat16
    inv = 1.0 / (10000.0 ** (np.arange(0, DH, 2, dtype=np.float32) / DH))
    t = np.arange(S, dtype=np.float32)
    fr = t[:, None] * inv[None, :]                       # [S, DH/2]
    cosT = np.repeat(np.cos(fr).astype(np.float32).T, 2, axis=0)  # [DH, S]
    sinT = np.repeat(np.sin(fr).astype(np.float32).T, 2, axis=0)

    # sliding causal mask: msk[k, c] = 0 if k <= c - 384 else MASK_NEG
    kk = np.arange(128)[:, None]
    cc = np.arange(896)[None, :]
    msk = np.where(kk <= cc - 384, 0.0, MASK_NEG).astype(np.float32)

    # pt = P.T with P@x the rotate-half shuffle
    pt = np.zeros((128, 128), dtype=np.float32)
    i = np.arange(0, 128, 2)
    pt[i + 1, i] = -1.0
    pt[i, i + 1] = 1.0

    identb = np.eye(128, dtype=np.float32).astype(BFNP)
    onescolb = np.ones((128, 1), dtype=np.float32).astype(BFNP)
    return cosT, sinT, msk, pt, identb, onescolb


def _run(inputs, trace=False, trace_kwargs=None):
    import ml_dtypes
    from concourse.bass_utils import run_bass_kernel_spmd

    BFNP = ml_dtypes.bfloat16

    if "nc" not in _CACHE:
        _CACHE["nc"] = _build_program()
    nc = _CACHE["nc"]

    resid_pre = np.asarray(inputs["resid_pre"], dtype=np.float32)
    W_Q = np.asarray(inputs["W_Q"], dtype=np.float32)
    W_K = np.asarray(inputs["W_K"], dtype=np.float32)
    W_V = np.asarray(inputs["W_V"], dtype=np.float32)
    W_O = np.asarray(inputs["W_O"], dtype=np.float32)
    b_Q = np.asarray(inputs["b_Q"], dtype=np.float32)
    b_K = np.asarray(inputs["b_K"], dtype=np.float32)
    b_V = np.asarray(inputs["b_V"], dtype=np.float32)
    b_O = np.asarray(inputs["b_O"], dtype=np.float32)

    xt = np.ascontiguousarray(resid_pre[0].T)
    cosT, sinT, msk, pt, identb, onescolb = _host_constants()

    in_maps = []
    for c in range(NC):
        hs = slice(c * HPC, (c + 1) * HPC)
        wl = np.empty((NDC, 6, 128, DH), dtype=np.float32)
        for dc in range(NDC):
            sl = slice(dc * 128, (dc + 1) * 128)
            wl[dc, 0] = W_Q[c * HPC + 0][sl]
            wl[dc, 1] = W_Q[c * HPC + 1][sl]
            wl[dc, 2] = W_K[c * HPC + 0][sl]
            wl[dc, 3] = W_K[c * HPC + 1][sl]
            wl[dc, 4] = W_V[c * HPC + 0][sl]
            wl[dc, 5] = W_V[c * HPC + 1][sl]
        in_maps.append({
            "xt": xt,
            "wall": wl,
            "wo": np.ascontiguousarray(W_O[hs]).astype(BFNP),
            "bq": np.ascontiguousarray(b_Q[hs][:, :, None]),
            "bk": np.ascontiguousarray(b_K[hs][:, :, None]),
            "bv": np.ascontiguousarray(b_V[hs][:, :, None]),
            "cost": cosT, "sint": sinT, "maskneg": msk, "pt": pt,
            "identb": identb, "onescolb": onescolb,
        })

    kw = dict(trace_kwargs or {})
    last_err = None
    for attempt in range(3):
        try:
            res = run_bass_kernel_spmd(nc, in_maps, list(range(NC)), trace=trace, **kw)
            break
        except Exception as e:  # transient NRT_EXEC_UNIT_UNRECOVERABLE wedges clear on retry
            last_err = e
            if attempt == 2 or "UNRECOVERABLE" not in str(e).upper() and "UNAVAILABLE" not in str(e).upper():
                raise
            import time
            time.sleep(3.0)
    else:
        raise last_err

    acc = np.zeros((D, S), dtype=np.float32)
    for c in range(NC):
        acc += np.asarray(res.results[c]["outt"]).astype(np.float32)
    out = acc.T + b_O[None, :]
    return out.reshape(1, S, D).astype(np.float32), res


def kernel(**inputs) -> np.ndarray:
    out, _ = _run(inputs, trace=False)
    return out


# revision 24
# speedup vs baseline: 1.0372x; 1.0372x over previous
"""Trainium2 Bass kernel for nn_Attention_10015863734775.

Multi-head causal attention (16 heads, d_model 2048, d_head 128, seq 2048,
batch 1) with llama-style interleaved RoPE and a signed-softmax:
    attn_w = sign(s) * exp(|s| - max|s|);  attn = attn_w / (sum|attn_w| + 1e-6)
The max-subtraction cancels in the normalization, so the device computes
attn = sign(s)exp(|s|) / sum exp(|s|).

Sharding: 2 heads per NeuronCore (8 cores). Each core receives the full
transposed residual X^T plus its head slices of W_Q/K/V/W_O and computes a
partial output projection outT_c[m, s] in bf16; the host sums the 8 partials
in f32, transposes, and adds b_O.

Precision split (empirically validated): q/k-path rounding amplifies ~15x
into the output (correlated per-row errors accumulate over the d_head
contraction), so X, W_Q/K/V, RoPE tables and qrot/krot stay f32/f32r.
Value-path tensors (e2, w, v, znt, W_O, output staging) are bf16 — each
contributes only ~0.2% rel-l2 — enabling bf16 z/denominator/output matmuls
(same 1 cycle/row as f32r) and moving the w=sign*e2 multiply to the
(otherwise idle, PSUM-incapable) GPSIMD engine.

Engine budget per score chunk: abs+sign (PSUM-reading: DVE/ACT), exp (ACT,
bf16 out), mult (GPSIMD/DVE-2x), causal mask adds (GPSIMD on SBUF |s|).
Denominator: ones-bf16 matmul accumulated per row; reciprocal_approx_fast
(~5x faster than reciprocal) + gpsimd partition_broadcast for 1/d.
"""

import math

import numpy as np

S = 2048          # sequence length
D = 2048          # d_model
DH = 128          # d_head
NH = 16           # total heads
NC = 8            # neuron cores
HPC = NH // NC    # heads per core (2)
ST = 512          # seq tile (matmul free dim / one PSUM bank)
NST = S // ST     # 4 seq tiles
NDC = D // 128    # 16 contraction chunks
NKC = S // 128    # 16 k chunks
C_SCALE = 1.0 / math.sqrt(float(DH))
LN2 = math.log(2.0)
MASK_NEG = -1.0e5

_CACHE = {}


def _build_program():
    import concourse.tile as tile
    from concourse import bacc, mybir

    F32 = mybir.dt.float32
    F32R = mybir.dt.float32r
    BF16 = mybir.dt.bfloat16
    U32 = mybir.dt.uint32
    AF = mybir.ActivationFunctionType
    ALU = mybir.AluOpType

    nc = bacc.Bacc("TRN2", target_bir_lowering=False, debug=False, num_devices=NC)

    xt_d = nc.dram_tensor("xt", [D, S], F32, kind="ExternalInput").ap()
    wall_d = nc.dram_tensor("wall", [NDC, 6, 128, DH], F32, kind="ExternalInput").ap()
    wo_d = nc.dram_tensor("wo", [HPC, DH, D], BF16, kind="ExternalInput").ap()
    bq_d = nc.dram_tensor("bq", [HPC, DH, 1], F32, kind="ExternalInput").ap()
    bk_d = nc.dram_tensor("bk", [HPC, DH, 1], F32, kind="ExternalInput").ap()
    bv_d = nc.dram_tensor("bv", [HPC, DH, 1], F32, kind="ExternalInput").ap()
    cos_d = nc.dram_tensor("cost", [DH, S], F32, kind="ExternalInput").ap()
    sin_d = nc.dram_tensor("sint", [DH, S], F32, kind="ExternalInput").ap()
    msk_d = nc.dram_tensor("maskneg", [128, 896], F32, kind="ExternalInput").ap()
    pt_d = nc.dram_tensor("pt", [128, 128], F32, kind="ExternalInput").ap()
    idb_d = nc.dram_tensor("identb", [128, 128], BF16, kind="ExternalInput").ap()
    ocb_d = nc.dram_tensor("onescolb", [128, 1], BF16, kind="ExternalInput").ap()
    out_d = nc.dram_tensor("outt", [D, S], BF16, kind="ExternalOutput").ap()

    with tile.TileContext(nc) as tc:
        with tc.tile_pool(name="persist", bufs=1) as pp:
            # ---------------- persistent SBUF ------------------------------
            bqs, bks, bvs = [], [], []
            qrot, krot, vb_sb, zntb = [], [], [], []
            for h in range(HPC):
                for lst, dd, nm in ((bqs, bq_d, "bq"), (bks, bk_d, "bk"), (bvs, bv_d, "bv")):
                    bt = pp.tile([DH, 1], F32, tag=f"{nm}{h}")
                    nc.scalar.dma_start(bt[:], dd[h])
                    lst.append(bt)
                qrot.append(pp.tile([DH, S], F32R, tag=f"qrot{h}", name=f"qrot{h}"))
                krot.append(pp.tile([DH, S], F32R, tag=f"krot{h}", name=f"krot{h}"))
                vb_sb.append(pp.tile([128, NKC, DH], BF16, tag=f"vb{h}", name=f"vb{h}"))
                zntb.append(pp.tile([DH, S], BF16, tag=f"zntb{h}", name=f"zntb{h}"))
            wo_sb = [pp.tile([DH, D], BF16, tag=f"wo{h}", name=f"wo{h}")
                     for h in range(HPC)]
            msk_sb = pp.tile([128, 896], F32, tag="msk")
            pt_sb = pp.tile([128, 128], F32R, tag="pt")
            nc.scalar.dma_start(pt_sb[:], pt_d[:].bitcast(F32R))
            idb_sb = pp.tile([128, 128], BF16, tag="identb")
            nc.scalar.dma_start(idb_sb[:], idb_d[:])
            ocb_sb = pp.tile([128, 1], BF16, tag="onescolb")
            nc.scalar.dma_start(ocb_sb[:], ocb_d[:])
            cabs_sb = pp.tile([128, 1], U32, tag="cabs")
            nc.vector.memset(cabs_sb[:], 0x7FFFFFFF)
            csgn_sb = pp.tile([128, 1], U32, tag="csgn")
            nc.vector.memset(csgn_sb[:], 0x80000000)
            cone_sb = pp.tile([128, 1], U32, tag="cone")
            nc.vector.memset(cone_sb[:], 0x3F800000)
            ln2_sb = pp.tile([128, 1], F32, tag="ln2")
            nc.vector.memset(ln2_sb[:], LN2)
            zero_sb = pp.tile([128, 1], F32, tag="zero")
            nc.vector.memset(zero_sb[:], 0.0)
            import os as _os
            _nonce = float(int(_os.environ.get("KBUILD_NONCE", "0")))
            nonce_sb = pp.tile([128, 1], F32, tag="nonce")
            nc.vector.memset(nonce_sb[:], _nonce)

            # greedy engine balancing for elementwise passes
            load = {"dve": 0.0, "act": 0.0, "gp": 0.0}

            def pick(*opts):
                e, c = min(opts, key=lambda ec: load[ec[0]] + ec[1])
                load[e] += c
                return e

            # ---------------- Phase A: projections + RoPE + V transpose ----
            with tc.tile_pool(name="aphase", bufs=1) as ap_, \
                 tc.tile_pool(name="axt", bufs=4) as axt, \
                 tc.tile_pool(name="aev", bufs=6) as aev, \
                 tc.tile_pool(name="arope", bufs=6) as arp, \
                 tc.tile_pool(name="psA", bufs=1, space="PSUM") as psA, \
                 tc.tile_pool(name="psShuf", bufs=1, space="PSUM") as psSh, \
                 tc.tile_pool(name="psVtr", bufs=1, space="PSUM") as psVt:
                wall_sb = ap_.tile([128, NDC, 6, DH], F32R, tag="wall", name="wall_sb")
                widx = {("q", 0): 0, ("q", 1): 1, ("k", 0): 2, ("k", 1): 3,
                        ("v", 0): 4, ("v", 1): 5}
                cos_sb = ap_.tile([DH, S], F32, tag="cos")
                sin_sb = ap_.tile([DH, S], F32, tag="sin")

                a_deferred = []
                for st in range(NST):
                    ssl = slice(st * ST, (st + 1) * ST)
                    acc = {}
                    for key in ("q", "k", "v"):
                        for h in range(HPC):
                            acc[(key, h)] = psA.tile([128, ST], F32, tag=f"acc{key}{h}",
                                                     name=f"acc{key}{h}_{st}")
                    for dc in range(NDC):
                        if st == 0:
                            nc.scalar.dma_start(
                                wall_sb[:, dc, :, :],
                                wall_d[dc].rearrange("i p e -> p i e").bitcast(F32R))
                        xt_t = axt.tile([128, ST], F32R, tag="xt")
                        nc.sync.dma_start(
                            xt_t[:],
                            xt_d[dc * 128:(dc + 1) * 128, ssl].bitcast(F32R))
                        if st == 0 and dc == 4:
                            nc.scalar.dma_start(cos_sb[:], cos_d[:])
                            nc.scalar.dma_start(sin_sb[:], sin_d[:])
                        for key in ("q", "k", "v"):
                            for h in range(HPC):
                                nc.tensor.matmul(
                                    acc[(key, h)][:], wall_sb[:, dc, widx[(key, h)], :],
                                    xt_t[:], start=(dc == 0), stop=(dc == NDC - 1))
                        if a_deferred and dc % 2 == 1:
                            a_deferred.pop(0)()

                    def make_rope(key, h, st, ssl, x_sb):
                        def run():
                            dst = (qrot if key == "q" else krot)[h]
                            shuf = psSh.tile([128, ST], F32, tag="shuf",
                                             name=f"sh{key}{h}_{st}")
                            nc.tensor.matmul(shuf[:], pt_sb[:], x_sb[:],
                                             start=True, stop=True)
                            t1 = arp.tile([128, ST], F32, tag="t1", name=f"t1{key}{h}_{st}")
                            if pick(("gp", 0.92), ("act", 0.52)) == "gp":
                                nc.gpsimd.tensor_tensor(t1[:], x_sb[:].bitcast(F32),
                                                        cos_sb[:, ssl], ALU.mult)
                            else:
                                # ACT has no tensor*tensor; route via DVE instead
                                nc.vector.tensor_tensor(t1[:], x_sb[:].bitcast(F32),
                                                        cos_sb[:, ssl], ALU.mult)
                            t2 = arp.tile([128, ST], F32, tag="t2", name=f"t2{key}{h}_{st}")
                            nc.vector.tensor_tensor(t2[:], shuf[:], sin_sb[:, ssl], ALU.mult)
                            nc.vector.tensor_tensor(dst[:, ssl], t1[:], t2[:], ALU.add)
                        return run

                    def make_vtr(h, st, vt_sb):
                        def run():
                            for sc in range(ST // 128):
                                vtr = psVt.tile([128, 128], BF16, tag="vtr",
                                                name=f"vtr{h}_{st}_{sc}")
                                nc.tensor.transpose(vtr[:],
                                                    vt_sb[:, sc * 128:(sc + 1) * 128],
                                                    idb_sb[:])
                                if pick(("dve", 0.17), ("act", 0.16)) == "dve":
                                    nc.vector.tensor_copy(vb_sb[h][:, st * 4 + sc, :],
                                                          vtr[:])
                                else:
                                    nc.scalar.activation(vb_sb[h][:, st * 4 + sc, :],
                                                         vtr[:], AF.Identity,
                                                         bias=zero_sb[:])
                        return run

                    for key in ("q", "k"):
                        for h in range(HPC):
                            bias = (bqs if key == "q" else bks)[h]
                            x_sb = aev.tile([128, ST], F32R, tag="ev", bufs=8,
                                            name=f"ev{key}{h}_{st}")
                            nc.scalar.activation(x_sb[:], acc[(key, h)][:], AF.Identity,
                                                 bias=bias[:])
                            a_deferred.append(make_rope(key, h, st, ssl, x_sb))
                    for h in range(HPC):
                        vt_sb = aev.tile([128, ST], BF16, tag="evv", bufs=4,
                                         name=f"evv{h}_{st}")
                        nc.scalar.activation(vt_sb[:], acc[("v", h)][:], AF.Identity,
                                             bias=bvs[h][:])
                        a_deferred.append(make_vtr(h, st, vt_sb))
                while a_deferred:
                    a_deferred.pop(0)()

            # deferred non-critical DMAs (needed in B/C only)
            nc.scalar.dma_start(msk_sb[:], msk_d[:])
            for h in range(HPC):
                nc.scalar.dma_start(wo_sb[h][:], wo_d[h])

            # ---------------- Phases B + C --------------------------------
            with tc.tile_pool(name="bwork", bufs=6) as bw, \
                 tc.tile_pool(name="bbf", bufs=4) as bb, \
                 tc.tile_pool(name="bsmall", bufs=3) as bsm, \
                 tc.tile_pool(name="psS", bufs=2, space="PSUM") as psS, \
                 tc.tile_pool(name="psB", bufs=1, space="PSUM") as psB, \
                 tc.tile_pool(name="psD", bufs=1, space="PSUM") as psD:
                # PSUM banks: psS 2 ("s") + psB 4 ("z0","z1", "co"x2) + psD 2 ("d0","d1") = 8

                dma_engines = [nc.sync, nc.scalar, nc.sync]

                def emit_scores(j, kc, h):
                    jj = kc - 4 * j
                    lo = jj * 128 if 0 <= jj < 4 else 0
                    pss = psS.tile([128, ST], F32, tag="s", name=f"s{h}_{j}_{kc}")
                    nc.tensor.matmul(pss[:, lo:], krot[h][:, kc * 128:(kc + 1) * 128],
                                     qrot[h][:, j * ST + lo:(j + 1) * ST],
                                     start=True, stop=True)
                    return {"j": j, "kc": kc, "h": h, "lo": lo, "pss": pss}

                def emit_elem(stt):
                    j, kc, h, lo, pss = stt["j"], stt["kc"], stt["h"], stt["lo"], stt["pss"]
                    jj = kc - 4 * j
                    lsl = slice(lo, ST)
                    f = (ST - lo) / ST
                    a = bw.tile([128, ST], F32, tag="a", name=f"a{h}_{j}_{kc}")
                    if pick(("dve", 0.60 * f), ("act", 0.52 * f)) == "dve":
                        nc.vector.tensor_scalar(a[:, lsl].bitcast(U32),
                                                pss[:, lsl].bitcast(U32),
                                                cabs_sb[:], None, ALU.bitwise_and)
                        exp_scale = C_SCALE
                    else:
                        nc.scalar.activation(a[:, lsl], pss[:, lsl], AF.Abs,
                                             bias=zero_sb[:], scale=C_SCALE)
                        exp_scale = 1.0
                    if 0 <= jj < 4:
                        # causal mask on |s|: -1e5 -> exp == 0 (gpsimd, SBUF)
                        load["gp"] += 0.25
                        nc.gpsimd.tensor_tensor(a[:, lo:lo + 128], a[:, lo:lo + 128],
                                                msk_sb[:, 384:512], ALU.add)
                    sg_bf = None
                    sg_f = None
                    if pick(("dve", 0.60 * f), ("act", 0.52 * f)) == "dve":
                        sg_f = bw.tile([128, ST], F32, tag="sg", name=f"sg{h}_{j}_{kc}")
                        nc.vector.tensor_scalar(sg_f[:, lsl].bitcast(U32),
                                                pss[:, lsl].bitcast(U32),
                                                csgn_sb[:], cone_sb[:],
                                                ALU.bitwise_and, ALU.bitwise_or)
                    else:
                        sg_bf = bb.tile([128, ST], BF16, tag="sgb", name=f"sgb{h}_{j}_{kc}")
                        nc.scalar.activation(sg_bf[:, lsl], pss[:, lsl], AF.Sign,
                                             bias=zero_sb[:])
                    e2 = bb.tile([128, ST], BF16, tag="e2", name=f"e2{h}_{j}_{kc}")
                    load["act"] += 0.52 * f
                    nc.scalar.activation(e2[:, lsl], a[:, lsl], AF.Exp,
                                         bias=ln2_sb[:], scale=exp_scale)
                    w = bb.tile([128, ST], BF16, tag="w", name=f"w{h}_{j}_{kc}")
                    if sg_bf is not None:
                        # all-bf16 SBUF multiply: DVE 2x-eligible
                        if pick(("dve", 0.32 * f), ("gp", 0.92 * f)) == "dve":
                            nc.vector.tensor_tensor(w[:, lsl], sg_bf[:, lsl],
                                                    e2[:, lsl], ALU.mult)
                        else:
                            nc.gpsimd.tensor_tensor(w[:, lsl], sg_bf[:, lsl],
                                                    e2[:, lsl], ALU.mult)
                    else:
                        if pick(("dve", 0.60 * f), ("gp", 0.92 * f)) == "dve":
                            nc.vector.tensor_tensor(w[:, lsl], sg_f[:, lsl],
                                                    e2[:, lsl], ALU.mult)
                        else:
                            nc.gpsimd.tensor_tensor(w[:, lsl], sg_f[:, lsl],
                                                    e2[:, lsl], ALU.mult)
                    stt["e2"], stt["w"] = e2, w

                def emit_zd(stt, psz, psd_t, nkc_):
                    j, kc, h, lo = stt["j"], stt["kc"], stt["h"], stt["lo"]
                    lsl = slice(lo, ST)
                    nc.tensor.matmul(psd_t[:, lsl], ocb_sb[:], stt["e2"][:, lsl],
                                     start=(kc == 0), stop=(kc == nkc_ - 1))
                    nc.tensor.matmul(psz[:, lsl], vb_sb[h][:, kc, :], stt["w"][:, lsl],
                                     start=(kc == 0), stop=(kc == nkc_ - 1))

                def finalize(j, h, psz, psd_t):
                    jsl = slice(j * ST, (j + 1) * ST)
                    r_sb = bsm.tile([1, ST], F32, tag="rsb", name=f"r{h}_{j}")
                    load["dve"] += 0.75
                    nc.vector.reciprocal_approx_fast(r_sb[:], psd_t[:])
                    rb_sb = bw.tile([128, ST], F32, tag="a", name=f"rbs{h}_{j}")
                    load["gp"] += 0.95
                    nc.gpsimd.partition_broadcast(rb_sb[:], r_sb[:])
                    load["dve"] += 0.60
                    nc.vector.tensor_tensor(zntb[h][:, jsl], psz[:], rb_sb[:], ALU.mult)

                def make_C(j):
                    jsl = slice(j * ST, (j + 1) * ST)
                    acts = []
                    for mc in range(16):
                        def run(mc=mc):
                            co = psB.tile([128, ST], F32, tag="co", bufs=2,
                                          name=f"o{j}_{mc}")
                            for h in range(HPC):
                                nc.tensor.matmul(co[:],
                                                 wo_sb[h][:, mc * 128:(mc + 1) * 128],
                                                 zntb[h][:, jsl],
                                                 start=(h == 0), stop=(h == HPC - 1))
                            o_sb = bb.tile([128, ST], BF16, tag="osb", bufs=3,
                                           name=f"osb{j}_{mc}")
                            if pick(("dve", 0.60), ("act", 0.52)) == "act":
                                nc.scalar.activation(o_sb[:], co[:], AF.Identity,
                                                     bias=zero_sb[:])
                            else:
                                nc.vector.tensor_copy(o_sb[:], co[:])
                            dma_engines[mc % 3].dma_start(
                                out_d[mc * 128:(mc + 1) * 128, jsl], o_sb[:])
                        acts.append(run)
                    return acts

                c_deferred = []
                units = [(j, kc, h) for j in range(NST)
                         for kc in range(4 * (j + 1)) for h in range(HPC)]
                LOOK = 2
                ZLAG = 2
                inflight = {}
                psz = {}
                psd_t = {}

                def stage_zd(u):
                    stt = inflight.pop(u)
                    j, h = stt["j"], stt["h"]
                    nkc_ = 4 * (j + 1)
                    if stt["kc"] == 0:
                        psd_t[(j, h)] = psD.tile([1, ST], F32, tag=f"d{h}", name=f"d{h}_{j}")
                        psz[(j, h)] = psB.tile([128, ST], F32, tag=f"z{h}",
                                               name=f"z{h}_{j}")
                    emit_zd(stt, psz[(j, h)], psd_t[(j, h)], nkc_)
                    if stt["kc"] == nkc_ - 1:
                        finalize(j, h, psz.pop((j, h)), psd_t.pop((j, h)))
                        if h == HPC - 1:
                            c_deferred.extend(make_C(j))

                for u, (j, kc, h) in enumerate(units):
                    inflight[u] = emit_scores(j, kc, h)
                    if u >= LOOK:
                        emit_elem(inflight[u - LOOK])
                    if u >= LOOK + ZLAG:
                        stage_zd(u - LOOK - ZLAG)
                    if c_deferred:
                        c_deferred.pop(0)()
                n = len(units)
                for u in range(n - LOOK, n):
                    emit_elem(inflight[u])
                for u in range(n - LOOK - ZLAG, n):
                    stage_zd(u)
                    if c_deferred:
                        c_deferred.pop(0)()
                while c_deferred:
                    c_deferred.pop(0)()
    nc.compile()
    return nc


def _host_constants():
    import ml_dtypes
    BFNP = ml_dtypes.bfloat16
    inv = 1.0 / (10000.0 ** (np.arange(0, DH, 2, dtype=np.float32) / DH))
    t = np.arange(S, dtype=np.float32)
    fr = t[:, None] * inv[None, :]                       # [S, DH/2]
    cosT = np.repeat(np.cos(fr).astype(np.float32).T, 2, axis=0)  # [DH, S]
    sinT = np.repeat(np.sin(fr).astype(np.float32).T, 2, axis=0)

    # sliding causal mask: msk[k, c] = 0 if k <= c - 384 else MASK_NEG
    kk = np.arange(128)[:, None]
    cc = np.arange(896)[None, :]
    msk = np.where(kk <= cc - 384, 0.0, MASK_NEG).astype(np.float32)

    # pt = P.T with P@x the rotate-half shuffle
    pt = np.zeros((128, 128), dtype=np.float32)
    i = np.arange(0, 128, 2)
    pt[i + 1, i] = -1.0
    pt[i, i + 1] = 1.0

    identb = np.eye(128, dtype=np.float32).astype(BFNP)
    onescolb = np.ones((128, 1), dtype=np.float32).astype(BFNP)
    return cosT, sinT, msk, pt, identb, onescolb


def _run(inputs, trace=False, trace_kwargs=None):
    import ml_dtypes
    from concourse.bass_utils import run_bass_kernel_spmd

    BFNP = ml_dtypes.bfloat16

    if "nc" not in _CACHE:
        _CACHE["nc"] = _build_program()
    nc = _CACHE["nc"]

    resid_pre = np.asarray(inputs["resid_pre"], dtype=np.float32)
    W_Q = np.asarray(inputs["W_Q"], dtype=np.float32)
    W_K = np.asarray(inputs["W_K"], dtype=np.float32)
    W_V = np.asarray(inputs["W_V"], dtype=np.float32)
    W_O = np.asarray(inputs["W_O"], dtype=np.float32)
    b_Q = np.asarray(inputs["b_Q"], dtype=np.float32)
    b_K = np.asarray(inputs["b_K"], dtype=np.float32)
    b_V = np.asarray(inputs["b_V"], dtype=np.float32)
    b_O = np.asarray(inputs["b_O"], dtype=np.float32)

    xt = np.ascontiguousarray(resid_pre[0].T)
    cosT, sinT, msk, pt, identb, onescolb = _host_constants()

    in_maps = []
    for c in range(NC):
        hs = slice(c * HPC, (c + 1) * HPC)
        wl = np.empty((NDC, 6, 128, DH), dtype=np.float32)
        for dc in range(NDC):
            sl = slice(dc * 128, (dc + 1) * 128)
            wl[dc, 0] = W_Q[c * HPC + 0][sl]
            wl[dc, 1] = W_Q[c * HPC + 1][sl]
            wl[dc, 2] = W_K[c * HPC + 0][sl]
            wl[dc, 3] = W_K[c * HPC + 1][sl]
            wl[dc, 4] = W_V[c * HPC + 0][sl]
            wl[dc, 5] = W_V[c * HPC + 1][sl]
        in_maps.append({
            "xt": xt,
            "wall": wl,
            "wo": np.ascontiguousarray(W_O[hs]).astype(BFNP),
            "bq": np.ascontiguousarray(b_Q[hs][:, :, None]),
            "bk": np.ascontiguousarray(b_K[hs][:, :, None]),
            "bv": np.ascontiguousarray(b_V[hs][:, :, None]),
            "cost": cosT, "sint": sinT, "maskneg": msk, "pt": pt,
            "identb": identb, "onescolb": onescolb,
        })

    kw = dict(trace_kwargs or {})
    last_err = None
    for attempt in range(3):
        try:
            res = run_bass_kernel_spmd(nc, in_maps, list(range(NC)), trace=trace, **kw)
            break
        except Exception as e:  # transient NRT_EXEC_UNIT_UNRECOVERABLE wedges clear on retry
            last_err = e
            if attempt == 2 or "UNRECOVERABLE" not in str(e).upper() and "UNAVAILABLE" not in str(e).upper():
                raise
            import time
            time.sleep(3.0)
    else:
        raise last_err

    acc = np.zeros((D, S), dtype=np.float32)
    for c in range(NC):
        acc += np.asarray(res.results[c]["outt"]).astype(np.float32)
    out = acc.T + b_O[None, :]
    return out.reshape(1, S, D).astype(np.float32), res


def kernel(**inputs) -> np.ndarray:
    out, _ = _run(inputs, trace=False)
    return out
